# revision 13
# baseline (speedup 1.0000x reference)
"""Classical ray marcher (volume rendering) Bass kernel for 8 Trainium2 cores.

Problem: B=8, R=16384 rays, S=64 samples/ray, C=3 channels.
  dens   = softplus(densities); deltas = diff(depths), last = 1e10
  alpha  = 1 - exp(-deltas*dens); t = 1 - alpha + 1e-10
  trans  = cumprod(t) with leading 1 (per ray over S)
  w      = alpha * trans[:-1]
  rgb    = sum_s w*colors ; depth = sum_s w*depths ; ft = trans[-1]

Sharding: core i handles batch i. The host pre-transposes each core's
inputs to a samples-on-partitions layout ("layout B", dual-packed: SBUF
partition p = s + 64*h for ray-half h), which turns every S-direction op
into a TensorE matmul over the partition axis:

  delta   = D^T @ depths         (D = +-1 shift-difference matrix)
  cumsum  = L^T @ lt             (L = strictly-lower block triangular ones)
  rgb/dep = M_k^T @ (w*colors_k) (M_k = ones over each ray-half)

The cumprod is done in log space EXACTLY including the +1e-10:
  lt = log(exp(-p) + 1e-10) = Ln(1e-10*u + 1e-10),  u = exp(23.025851 - p)
so trans_excl = exp(L^T lt), and alpha = 1 - 1e-10*u, both straight from u.
Per-partition scale/bias vectors on the Ln/Identity activations pin the
s=63 rows (delta = 1e10) to their exact values without extra ops.

Engines: PE does delta/cumsum/reductions; ACT does softplus/exp/ln chains;
DVE does the three remaining elementwise products; GpSimd does w*depths.
"""

import numpy as np

import concourse.bacc as bacc
import concourse.mybir as mybir
from concourse import tile
from concourse.bass_utils import run_bass_kernel_spmd

F32 = mybir.dt.float32
AF = mybir.ActivationFunctionType
OP = mybir.AluOpType

B, R, S, C = 8, 16384, 64, 3
P = 128
HALF = R // 2          # rays per partition-half (dual-pack)
F = 1024               # rays (free dim) per tile
T = HALF // F          # 8 tiles per core
EPS = 1e-10
LNEPS = 23.025851      # -ln(1e-10)

N_CORES = 8
NCONST = 128 + 128 + 32 + 2 + 5   # D | L | M(4x8) | Lf(2) | scale_l,bias_l,scale_a,bias_a,bias_u

_BUILT = {}


class _Bacc(bacc.Bacc):
    """Bacc with a pinned activation-table choice: strip Exp/Ln from every
    table except 'natural_log_exp_and_others' (order/length unchanged, so
    act_func_set ids stay canonical) -> one table load, no ping-pong."""

    def insert_act_table_loads(self):
        from concourse.hw_specs import get_activation_tables
        import bass_rust as _br

        has_activation = any(
            isinstance(i, mybir.InstActivation)
            for b in self.main_func.blocks
            for i in b.instructions
        )
        if not has_activation:
            return
        keep = "natural_log_exp_and_others"
        strip = {AF.Exp, AF.Ln}
        tables = []
        for name, fns in get_activation_tables(self.m.arch).items():
            if name != keep:
                fns = set(fns) - strip
            tables.append((name, fns))
        _br.insert_act_table_loads(self, tables)


def _host_consts():
    D = np.zeros((P, P), np.float32)
    L = np.zeros((P, P), np.float32)
    for blk in (0, 64):
        for i2 in range(64):
            i = blk + i2
            if i2 < 63:
                D[i + 1, i] = 1.0
                D[i, i] = -1.0
            L[blk:blk + i2, i] = 1.0          # strictly lower: exclusive cumsum
    # each M_k is [128, 8] with ones at column 2k+h over partition-half h
    M = np.zeros((P, 32), np.float32)
    for k in range(4):                         # rgb0, rgb1, rgb2, depth
        for h in range(2):
            M[h * 64:(h + 1) * 64, 8 * k + 2 * k + h] = 1.0

    Lf = np.zeros((P, 2), np.float32)
    Lf[0:64, 0] = 1.0
    Lf[64:128, 1] = 1.0

    vecs = np.zeros((P, 5), np.float32)
    vecs[:, 0] = EPS      # scale_l
    vecs[:, 1] = EPS      # bias_l
    vecs[:, 2] = -EPS     # scale_a
    vecs[:, 3] = 1.0      # bias_a
    vecs[:, 4] = LNEPS    # bias_u
    vecs[63, 0] = 0.0
    vecs[127, 0] = 0.0    # lt rows 63/127 -> Ln(1e-10) exactly
    vecs[63, 2] = 0.0
    vecs[127, 2] = 0.0    # alpha rows 63/127 -> 1 exactly
    return np.concatenate([D, L, M, Lf, vecs], axis=1)


def _build():
    nc = _Bacc("TRN2", target_bir_lowering=False, debug=False)

    pack_d = nc.dram_tensor("pack", [T, 5, P, F], F32, kind="ExternalInput")
    cst_d = nc.dram_tensor("consts", [P, NCONST], F32, kind="ExternalInput")

    w_out_d = nc.dram_tensor("weights", [T, P, F], F32, kind="ExternalOutput")
    outs_d = nc.dram_tensor("outs", [T, 8, F], F32, kind="ExternalOutput")
    ft_d = nc.dram_tensor("ft", [T, 2, F], F32, kind="ExternalOutput")

    with tile.TileContext(nc) as tc:
        with (
            tc.tile_pool(name="const", bufs=1) as constp,
            tc.tile_pool(name="io", bufs=3) as io,
            tc.tile_pool(name="work", bufs=2) as work,
            tc.tile_pool(name="ps", bufs=1, space="PSUM") as ps,
        ):
            cst = constp.tile([P, NCONST], F32)
            nc.sync.dma_start(cst[:], cst_d.ap())
            Dm = cst[:, 0:128]
            Lm = cst[:, 128:256]
            Mm = cst[:, 256:288]
            Lfm = cst[:, 288:290]
            scale_l = cst[:, 290:291]
            bias_l = cst[:, 291:292]
            scale_a = cst[:, 292:293]
            bias_a = cst[:, 293:294]
            bias_u = cst[:, 294:295]

            for j in range(T):
                pack = io.tile([P, 5 * F], F32, tag="pack")
                nc.sync.dma_start(
                    pack.rearrange("p (t f) -> p t f", t=5),
                    pack_d.ap()[j].rearrange("t p f -> p t f"),
                )
                colv = [pack[:, c * F:(c + 1) * F] for c in range(C)]
                depv = pack[:, 3 * F:4 * F]
                denv = pack[:, 4 * F:5 * F]

                # dens = softplus(raw) = Ln(Exp(raw)+1)     [ACT]
                dens = work.tile([P, F], F32, tag="dens")
                nc.scalar.activation(dens[:], denv, AF.Exp)
                nc.scalar.activation(dens[:], dens[:], AF.Ln, bias=1.0)

                # delta = D^T @ depths  (rows 63/127 = 0, fixed up via lt/alpha)
                dpsum = ps.tile([P, F], F32, tag="dpsum", bufs=1)
                for hlf in range(2):
                    sl = slice(hlf * 512, (hlf + 1) * 512)
                    nc.tensor.matmul(dpsum[:, sl], Dm, depv[:, sl],
                                     start=True, stop=True)

                # p = delta * dens                          [DVE, PSUM src]
                p = work.tile([P, F], F32, tag="p")
                nc.vector.tensor_tensor(p[:], dpsum[:], dens[:], OP.mult)

                # u = exp(23.025851 - p)                    [ACT]
                u = work.tile([P, F], F32, tag="u")
                nc.scalar.activation(u[:], p[:], AF.Exp, scale=-1.0, bias=bias_u)

                # alpha = 1 - 1e-10*u (= 1 - exp(-p)); rows 63/127 -> 1
                alpha = work.tile([P, F], F32, tag="alpha")
                nc.scalar.activation(alpha[:], u[:], AF.Identity,
                                     scale=scale_a, bias=bias_a)

                # lt = Ln(1e-10*u + 1e-10) = log(exp(-p)+1e-10), in place on u
                nc.scalar.activation(u[:], u[:], AF.Ln,
                                     scale=scale_l, bias=bias_l)

                # exclusive cumsum of lt over samples: L^T @ lt   [PE]
                epsum = ps.tile([P, F], F32, tag="epsum", bufs=1)
                for hlf in range(2):
                    sl = slice(hlf * 512, (hlf + 1) * 512)
                    nc.tensor.matmul(epsum[:, sl], Lm, u[:, sl],
                                     start=True, stop=True)

                # trans_excl = exp(cumsum)                  [ACT, PSUM src]
                excl = work.tile([P, F], F32, tag="excl")
                nc.scalar.activation(excl[:], epsum[:], AF.Exp)

                # ft = exp(cumsum[63] + lt[63]) (inclusive last row per half)
                # ft = exp(full-block sum of lt) = full inclusive cumprod
                ftp = ps.tile([2, F], F32, tag="ftp", bufs=1)
                for hlf in range(2):
                    sl = slice(hlf * 512, (hlf + 1) * 512)
                    nc.tensor.matmul(ftp[:, sl], Lfm, u[:, sl],
                                     start=True, stop=True)
                ft_t = work.tile([2, F], F32, tag="ft_t")
                nc.scalar.activation(ft_t[:], ftp[:], AF.Exp)

                # w = alpha * trans_excl                    [DVE]
                w = work.tile([P, F], F32, tag="w")
                nc.vector.tensor_tensor(w[:], alpha[:], excl[:], OP.mult)

                # wc_c = w * colors_c (in place on pack)    [DVE]
                for c in range(C):
                    nc.vector.tensor_tensor(colv[c], colv[c], w[:], OP.mult)
                # wd = w * depths (in place on pack)        [GpSimd]
                nc.gpsimd.tensor_tensor(depv, depv, w[:], OP.mult)

                # rgb/depth reductions: accumulate M_k^T @ wc_k into [8, F]
                opsum = ps.tile([8, F], F32, tag="opsum", bufs=1)
                for hlf in range(2):
                    sl = slice(hlf * 512, (hlf + 1) * 512)
                    for k in range(4):
                        src = colv[k] if k < 3 else depv
                        nc.tensor.matmul(
                            opsum[:, sl], Mm[:, 8 * k:8 * (k + 1)], src[:, sl],
                            start=(k == 0), stop=(k == 3),
                        )
                outs_sb = work.tile([8, F], F32, tag="outs_sb")
                nc.scalar.copy(outs_sb[:], opsum[:])

                # stores
                nc.sync.dma_start(w_out_d.ap()[j], w[:])
                nc.scalar.dma_start(outs_d.ap()[j], outs_sb[:])
                nc.scalar.dma_start(ft_d.ap()[j], ft_t[:])

    nc.compile()
    return nc


def _get_nc():
    if "nc" not in _BUILT:
        _BUILT["nc"] = _build()
    return _BUILT["nc"]


def _run(in_maps, trace=False, **kw):
    nc = _get_nc()
    return run_bass_kernel_spmd(nc, in_maps, list(range(len(in_maps))), trace=trace, **kw)


def _to_layout_b(x):
    """[R, S] -> [128, HALF]: partition = s + 64*(ray >= HALF)."""
    return np.ascontiguousarray(
        x.reshape(2, HALF, S).transpose(0, 2, 1).reshape(P, HALF)
    )


def _prep_core(colors_i, densities_i, depths_i):
    dep_b = _to_layout_b(depths_i[:, :, 0])
    den_b = _to_layout_b(densities_i[:, :, 0])
    pack = np.empty((T, 5, P, F), np.float32)
    for c in range(C):
        cb = _to_layout_b(np.ascontiguousarray(colors_i[:, :, c]))
        pack[:, c] = cb.reshape(P, T, F).transpose(1, 0, 2)
    pack[:, 3] = dep_b.reshape(P, T, F).transpose(1, 0, 2)
    pack[:, 4] = den_b.reshape(P, T, F).transpose(1, 0, 2)
    return pack


def kernel(colors, densities, depths):
    """Full-input entry point: colors [8,16384,64,3], densities/depths [8,16384,64,1].

    Returns (rgb_final [B,R,C], depth [B,R,1], weights [B,R,S,1], final_trans [B,R]).
    """
    colors = np.ascontiguousarray(colors, dtype=np.float32)
    densities = np.ascontiguousarray(densities, dtype=np.float32)
    depths = np.ascontiguousarray(depths, dtype=np.float32)

    cst = _host_consts()
    in_maps = [
        {"pack": _prep_core(colors[i], densities[i], depths[i]), "consts": cst}
        for i in range(B)
    ]
    res = _run(in_maps).results

    rgb = np.empty((B, R, C), np.float32)
    depth = np.empty((B, R, 1), np.float32)
    weights = np.empty((B, R, S, 1), np.float32)
    ft = np.empty((B, R), np.float32)
    for i in range(B):
        wv = res[i]["weights"]                      # [T, 128, F]
        wb = wv.transpose(1, 0, 2).reshape(P, HALF)  # [128, HALF]
        weights[i, :, :, 0] = (
            wb.reshape(2, S, HALF).transpose(0, 2, 1).reshape(R, S)
        )
        outs = res[i]["outs"]                        # [T, 8, F]
        ob = outs.transpose(1, 0, 2).reshape(8, HALF)
        for c in range(C):
            for h in range(2):
                rgb[i, h * HALF:(h + 1) * HALF, c] = ob[2 * c + h]
        for h in range(2):
            depth[i, h * HALF:(h + 1) * HALF, 0] = ob[6 + h]
        fv = res[i]["ft"].transpose(1, 0, 2).reshape(2, HALF)
        for h in range(2):
            ft[i, h * HALF:(h + 1) * HALF] = fv[h]
    return rgb, depth, weights, ft


# revision 14
# speedup vs baseline: 1.1900x; 1.1900x over previous
"""Classical ray marcher (volume rendering) Bass kernel for 8 Trainium2 cores.

Problem: B=8, R=16384 rays, S=64 samples/ray, C=3 channels.
  dens   = softplus(densities)
  deltas = diff(depths) with last delta = 1e10
  alpha  = 1 - exp(-deltas*dens)
  t      = 1 - alpha + 1e-10
  trans  = cumprod(t) with leading 1          (per ray, over S)
  w      = alpha * trans[:-1]  ==  trans[s-1] - trans[s]   (up to 1e-10*trans)
  rgb    = sum_s w*colors ; depth = sum_s w*depths ; ft = trans[-1]

Sharding: core i handles batch i (embarrassingly parallel over rays).
On-chip layout: 128 rays on partitions, G rays x 64 samples on the free dim.
The per-ray cumprod runs as ONE hardware linear-recurrence scan per tile
(state = t*state + b), where b is zero except at each ray's first sample
(carries t[0]) - that resets the recurrence at ray boundaries.

Engine split per tile (load-balanced around the 1-elem/cycle fp32 DVE):
  GpSimd: delta sub, p=delta*dens, wd=w*depths, small memsets
  ACT:    exp/ln softplus, exp(-p), +eps, column fixups; store DMAs (HWDGE)
  DVE:    cumprod scan, w = c_prev - c, wc = w*colors, both reductions
  sync:   load DMAs (HWDGE)
"""

import numpy as np

import concourse.bacc as bacc
import concourse.mybir as mybir
from concourse import tile
from concourse.bass_utils import run_bass_kernel_spmd

F32 = mybir.dt.float32
AF = mybir.ActivationFunctionType
OP = mybir.AluOpType

B, R, S, C = 8, 16384, 64, 3
P = 128           # SBUF partitions (rays per partition-tile)
EPS = 1e-10

N_CORES = 8

_BUILT = {}


class _Bacc(bacc.Bacc):
    """Bacc with a pinned activation-table choice.

    Exp and Ln both live in the 'natural_log_exp_and_others' table, but the
    default table-choice pass assigns each activation the first table that
    contains its function, which ping-pongs Exp->exp_and_others /
    Ln->natural_log and inserts a ~1.3us ACT_TABLE_LOAD per switch. Strip
    Exp/Ln from every other table (list order and length unchanged, so
    act_func_set_ids stay canonical) so one table covers the whole kernel.
    """

    def insert_act_table_loads(self):
        from concourse.hw_specs import get_activation_tables
        import bass_rust as _br

        has_activation = any(
            isinstance(i, mybir.InstActivation)
            for b in self.main_func.blocks
            for i in b.instructions
        )
        if not has_activation:
            return
        keep = "natural_log_exp_and_others"
        strip = {AF.Exp, AF.Ln}
        tables = []
        for name, fns in get_activation_tables(self.m.arch).items():
            if name != keep:
                fns = set(fns) - strip
            tables.append((name, fns))
        _br.insert_act_table_loads(self, tables)


def _build(n_rays: int, g: int):
    """Build the single-core Bass module for n_rays rays, g rays/partition/tile."""
    nc = _Bacc("TRN2", target_bir_lowering=False, debug=False)

    w_free = g * S            # free width of S-sized tiles
    wc_free = g * S * C       # free width of color tiles
    rays_per_tile = P * g
    n_tiles = n_rays // rays_per_tile
    assert n_tiles * rays_per_tile == n_rays

    colors_d = nc.dram_tensor("colors", [n_rays, S * C], F32, kind="ExternalInput")
    dens_d = nc.dram_tensor("densities", [n_rays, S], F32, kind="ExternalInput")
    depths_d = nc.dram_tensor("depths", [n_rays, S], F32, kind="ExternalInput")

    w_out_d = nc.dram_tensor("weights", [n_rays, S], F32, kind="ExternalOutput")
    rgb_d = nc.dram_tensor("rgb", [n_rays, C], F32, kind="ExternalOutput")
    depth_d = nc.dram_tensor("depth_out", [n_rays], F32, kind="ExternalOutput")
    ft_d = nc.dram_tensor("ft", [n_rays], F32, kind="ExternalOutput")

    with tile.TileContext(nc) as tc:
        with (
            tc.tile_pool(name="const", bufs=1) as constp,
            tc.tile_pool(name="io", bufs=3) as io,
            tc.tile_pool(name="work", bufs=2) as work,
        ):
            # scan's additive input: all zeros except each ray's s=0 column,
            # which is rewritten per tile with that ray's t[0].
            b1 = constp.tile([P, w_free], F32)
            nc.vector.memset(b1[:], 0.0)

            for i in range(n_tiles):
                rays = slice(i * rays_per_tile, (i + 1) * rays_per_tile)

                col_t = io.tile([P, wc_free], F32, tag="col")
                dep_t = io.tile([P, w_free], F32, tag="dep")
                den_t = io.tile([P, w_free], F32, tag="den")
                nc.sync.dma_start(
                    col_t[:], colors_d.ap()[rays].rearrange("(p g) w -> p (g w)", p=P)
                )
                nc.sync.dma_start(
                    dep_t[:], depths_d.ap()[rays].rearrange("(p g) w -> p (g w)", p=P)
                )
                nc.sync.dma_start(
                    den_t[:], dens_d.ap()[rays].rearrange("(p g) w -> p (g w)", p=P)
                )

                dep3 = dep_t.rearrange("p (g s) -> p g s", g=g)

                # dens = softplus(raw) = Ln(Exp(raw)+1)   [ACT x2, one table]
                dens = work.tile([P, w_free], F32, tag="dens")
                nc.scalar.activation(dens[:], den_t[:], AF.Exp)
                nc.scalar.activation(dens[:], dens[:], AF.Ln, bias=1.0)
                dens3 = dens.rearrange("p (g s) -> p g s", g=g)

                # X: delta -> p -> e -> t, all in place
                x = work.tile([P, w_free], F32, tag="x")
                x3 = x.rearrange("p (g s) -> p g s", g=g)
                nc.vector.tensor_tensor(
                    x3[:, :, 0:S - 1], dep3[:, :, 1:S], dep3[:, :, 0:S - 1],
                    OP.subtract,
                )
                nc.vector.tensor_tensor(
                    x3[:, :, 0:S - 1], x3[:, :, 0:S - 1], dens3[:, :, 0:S - 1],
                    OP.mult,
                )
                nc.scalar.activation(
                    x3[:, :, 0:S - 1], x3[:, :, 0:S - 1], AF.Exp, scale=-1.0
                )
                nc.scalar.activation(
                    x3[:, :, 0:S - 1], x3[:, :, 0:S - 1], AF.Copy, bias=EPS
                )
                # t[63] = exp(-1e10*dens)+eps = eps exactly for any sane dens
                nc.gpsimd.memset(x3[:, :, S - 1:S], EPS)

                # segment-reset plumbing: b1[s=0] = t[0]; then t[0] := 0
                b13 = b1.rearrange("p (g s) -> p g s", g=g)
                nc.scalar.copy(b13[:, :, 0:1], x3[:, :, 0:1])
                nc.gpsimd.memset(x3[:, :, 0:1], 0.0)

                # c[s] = inclusive cumprod of t per ray    [DVE scan]
                c = work.tile([P, w_free], F32, tag="c")
                nc.vector.tensor_tensor_scan(
                    c[:], x[:], b1[:], 0.0, OP.mult, OP.add
                )
                c3 = c.rearrange("p (g s) -> p g s", g=g)

                # w[0] = 1 - c[0]; w[s] = c[s-1] - c[s]  (== alpha*trans, +-1e-10*c)
                w = work.tile([P, w_free], F32, tag="w")
                w3 = w.rearrange("p (g s) -> p g s", g=g)
                nc.scalar.activation(
                    w3[:, :, 0:1], c3[:, :, 0:1], AF.Copy, bias=1.0, scale=-1.0
                )
                nc.vector.tensor_tensor(
                    w3[:, :, 1:S], c3[:, :, 0:S - 1], c3[:, :, 1:S], OP.subtract
                )

                # wc = w (broadcast over C) * colors, written [g, c, s]-major
                # so the rgb reduction reads a contiguous inner axis [DVE]
                wc = work.tile([P, wc_free], F32, tag="wc")
                wc4 = wc.rearrange("p (g s c) -> p g s c", g=g, s=S)
                col4 = col_t.rearrange("p (g s c) -> p g s c", g=g, s=S)
                wbc = w3.unsqueeze(3).broadcast_to([P, g, S, C])
                nc.vector.tensor_tensor(wc4[:], col4[:], wbc, OP.mult)

                # rgb[g,c] = sum_s wc[g,s,c]   [DVE reduce over strided S view]
                rgb_t = work.tile([P, g * C], F32, tag="rgb_t")
                rgb3 = rgb_t.rearrange("p (g c) -> p g c", g=g)
                nc.vector.tensor_reduce(
                    rgb3[:], wc4.transpose([0, 1, 3, 2]), mybir.AxisListType.X, OP.add
                )

                # wd = w * depths (into X, dead after the scan)  [GpSimd]
                nc.gpsimd.tensor_tensor(x[:], w[:], dep_t[:], OP.mult)
                dep_o = work.tile([P, g], F32, tag="dep_o")
                nc.vector.tensor_reduce(
                    dep_o[:], x3[:], mybir.AxisListType.X, OP.add
                )

                # ft = c[63] (full cumprod incl. the 1e-10 last factor)
                ft_t = work.tile([P, g], F32, tag="ft_t")
                nc.scalar.copy(ft_t[:].unsqueeze(2), c3[:, :, S - 1:S])

                # stores on the ACT HWDGE ring (loads use the sync ring)
                nc.scalar.dma_start(
                    w_out_d.ap()[rays].rearrange("(p g) s -> p (g s)", p=P), w[:]
                )
                nc.scalar.dma_start(
                    rgb_d.ap()[rays].rearrange("(p g) c -> p (g c)", p=P), rgb_t[:]
                )
                nc.scalar.dma_start(
                    depth_d.ap()[rays].rearrange("(p g) -> p g", p=P), dep_o[:]
                )
                nc.scalar.dma_start(
                    ft_d.ap()[rays].rearrange("(p g) -> p g", p=P), ft_t[:]
                )

    nc.compile()
    return nc


def _get_nc(n_rays=R, g=16):
    key = (n_rays, g)
    if key not in _BUILT:
        _BUILT[key] = _build(n_rays, g)
    return _BUILT[key]


def _run(in_maps, n_rays=R, g=16, trace=False, **kw):
    nc = _get_nc(n_rays, g)
    return run_bass_kernel_spmd(nc, in_maps, list(range(len(in_maps))), trace=trace, **kw)


def kernel(colors, densities, depths):
    """Full-input entry point: colors [8,16384,64,3], densities/depths [8,16384,64,1].

    Returns (rgb_final [B,R,C], depth [B,R,1], weights [B,R,S,1], final_trans [B,R]).
    """
    colors = np.ascontiguousarray(colors, dtype=np.float32)
    densities = np.ascontiguousarray(densities, dtype=np.float32)
    depths = np.ascontiguousarray(depths, dtype=np.float32)

    in_maps = [
        {
            "colors": colors[i].reshape(R, S * C),
            "densities": densities[i].reshape(R, S),
            "depths": depths[i].reshape(R, S),
        }
        for i in range(B)
    ]
    res = _run(in_maps).results

    rgb = np.stack([res[i]["rgb"] for i in range(B)])                    # [B,R,C]
    depth = np.stack([res[i]["depth_out"] for i in range(B)])[..., None]  # [B,R,1]
    weights = np.stack([res[i]["weights"] for i in range(B)])[..., None]  # [B,R,S,1]
    ft = np.stack([res[i]["ft"] for i in range(B)])                      # [B,R]
    return rgb, depth, weights, ft
